# revision 30
# baseline (speedup 1.0000x reference)
import sys, os, math
sys.path.insert(0, "/opt/trn_rl_repo")
import numpy as np
import ml_dtypes

import concourse.bass as bass
import concourse.mybir as mybir
import concourse.tile as tile
from concourse import bacc
from concourse.bass_utils import run_bass_kernel_spmd

BF16 = mybir.dt.bfloat16
F32 = mybir.dt.float32
AF = mybir.ActivationFunctionType
ALU = mybir.AluOpType

D = 2048; S = 2048; H = 16; DH = 128; DF = 8192
EPS = 1.1920929e-07
NB = 16          # d-blocks of 128
SC = 4           # s-chunks of 512
QH = 2           # q-halves of 1024
bf = ml_dtypes.bfloat16

SKIP_T = 3.0     # skip off-diag score tile when s_min_slot * d_min > SKIP_T
LPS_DVE_WINS = set()   # probs-window indices whose softmax-denominator sum runs on DVE


def _present_kbs(t, qh):
    """kb blocks computed for head-slot t, q-window qh (static, all cores).
    Slot t's minimum slope across cores is 2^-(2t+2) (head 4t+3)."""
    smin = 2.0 ** (-(2 * t + 2))
    base = 8 * qh
    out = []
    for kb in range(NB):
        if kb < base:
            dmin = 128 * (base - kb) - 127
        elif kb >= base + 8:
            dmin = 128 * (kb - base) - 1023
        else:
            dmin = 0
        if smin * dmin <= SKIP_T:
            out.append(kb)
    return out

_NC = None
LAST_EXEC_NS = None


def _build():
    nc = bacc.Bacc("TRN2", target_bir_lowering=False, debug=False)

    dram = {}
    def din(name, shape, dt=BF16):
        dram[name] = nc.dram_tensor(name, list(shape), dt, kind="ExternalInput").ap()
        return dram[name]
    def dout(name, shape, dt=BF16):
        dram[name] = nc.dram_tensor(name, list(shape), dt, kind="ExternalOutput").ap()
        return dram[name]

    xT_d   = din("xT",   [NB, 128, S])            # xT[db,p,s] = x[g].T
    wqk_d  = din("wqk",  [8, 128, NB, 128])       # q(4 heads)+k(4 heads) lhsT tiles
    wv_d   = din("wv",   [128, NB, 512])          # v weights, rhs layout
    wg_d   = din("wg",   [4, 128, NB, 128])
    wup_d  = din("wup",  [32, 128, NB, 128])      # 16 u1-blocks then 16 u2-blocks
    wdw_d  = din("wdw",  [16, 2, 128, 8, 128])    # [ob, fhalf, p(f), fb, dout]
    wout_d = din("wout", [16, 128, 4, 128])       # [ob, p(c), cb, dout]
    flo_d  = din("flo",  [128, 4, 1024])          # e^{-s*t} decay masters (all parts same)
    fhi_d  = din("fhi",  [128, 4, 1024])          # e^{-s*(1023-t)}
    dgh_d  = din("dgh",  [128, 4, 896])           # banded diag bias per head
    bexp_d = din("bexp", [128, 4, 28], F32)       # per-partition exp bias table
    bqk_d  = din("bqk",  [128, 8], F32)
    bg_d   = din("bg",   [128, 4], F32)
    bup_d  = din("bup",  [128, 32], F32)
    bv_d   = din("bv",   [128, 4], F32)

    ao_d  = dout("attn_outT", [16, 128, S])       # [ob, p(dout), s]
    fa_d  = dout("ffn_aT",   [16, 128, S])
    fb_d  = dout("ffn_bT",   [16, 128, S])

    with tile.TileContext(nc) as tc:
        with tc.tile_pool(name="const", bufs=1) as constp, \
             tc.tile_pool(name="ev", bufs=4) as evp, \
             tc.tile_pool(name="xnp", bufs=1) as xnp, \
             tc.tile_pool(name="ffS5", bufs=3) as ffw:

            ones128 = constp.tile([128, 128], BF16)
            nc.vector.memset(ones128[:], 1.0)
            ones1 = constp.tile([1, 128], BF16)
            nc.vector.memset(ones1[:], 1.0)
            epst = constp.tile([128, 1], F32)
            nc.vector.memset(epst[:], EPS)
            bqk = constp.tile([128, 8], F32)
            bg = constp.tile([128, 4], F32)
            bup = constp.tile([128, 32], F32)
            bv = constp.tile([128, 4], F32)

            xn = xnp.tile([128, NB, S], BF16)       # stays resident to end of FFN

            with tc.tile_pool(name="qkp", bufs=1) as qkp, \
                 tc.tile_pool(name="gatep", bufs=1) as gatep:
                qkT = qkp.tile([128, 8, S], BF16)
                vsb = qkp.tile([128, NB, 512], BF16)
                gateT = gatep.tile([128, 4, S], BF16)

                # ============ phase A: RMSNorm + qkv (sc-pipelined) ============
                with tc.tile_pool(name="wvp", bufs=1) as wvgp:
                    wvt = wvgp.tile([128, NB, 512], BF16)

                    with tc.tile_pool(name="wqk", bufs=1) as wqkp, \
                         tc.tile_pool(name="p0", bufs=1) as p0, \
                         tc.tile_pool(name="psB", bufs=1, space="PSUM") as psB, \
                         tc.tile_pool(name="ps1", bufs=4, space="PSUM") as ps:

                        def dma_x(sc):
                            for db in range(NB):
                                nc.sync.dma_start(out=xn[:, db, sc*512:(sc+1)*512],
                                                  in_=xT_d[db, :, sc*512:(sc+1)*512])

                        wqk_all = wqkp.tile([128, 8, NB, 128], BF16)

                        dma_x(0)
                        dma_x(1)
                        # p-state warmup: keep the PE busy while the first x
                        # chunk lands so real matmuls start at full clock
                        warm = psB.tile([128, 128], F32, tag="warm", bufs=1)
                        for _ in range(40):
                            nc.tensor.matmul(out=warm[:], lhsT=ones128[:],
                                             rhs=ones128[:], start=True, stop=True)
                        nc.sync.dma_start(out=bqk[:], in_=bqk_d[:, :])
                        nc.sync.dma_start(out=bg[:], in_=bg_d[:, :])
                        nc.sync.dma_start(out=bup[:], in_=bup_d[:, :])
                        nc.sync.dma_start(out=bv[:], in_=bv_d[:, :])
                        for cb in range(8):
                            nc.sync.dma_start(out=wqk_all[:, cb, :, :],
                                              in_=wqk_d[cb, :, :, :])
                        dma_x(2)
                        dma_x(3)
                        nc.sync.dma_start(out=wvt[:], in_=wv_d[:, :, :])

                        def rms(sc):
                            ss = slice(sc*512, (sc+1)*512)
                            ms = psB.tile([128, 512], F32, tag="ms", bufs=2)
                            for db in range(NB):
                                xsq = p0.tile([128, 512], BF16, tag="xsq", bufs=1)
                                nc.vector.tensor_tensor(out=xsq[:], in0=xn[:, db, ss],
                                                        in1=xn[:, db, ss], op=ALU.mult)
                                nc.tensor.matmul(out=ms[:], lhsT=ones128[:], rhs=xsq[:],
                                                 start=(db == 0), stop=(db == NB - 1))
                            rs = p0.tile([128, 512], F32, tag="rs", bufs=1)
                            nc.scalar.activation(rs[:], ms[:], AF.Sqrt, bias=epst[:],
                                                 scale=1.0 / D)
                            nc.vector.reciprocal_approx_fast(out=rs[:], in_=rs[:])
                            for db in range(NB):
                                nc.vector.tensor_tensor(out=xn[:, db, ss], in0=xn[:, db, ss],
                                                        in1=rs[:], op=ALU.mult)

                        def qkv_cols(cb, w_ap, sc):
                            ss = slice(sc*512, (sc+1)*512)
                            p = ps.tile([128, 512], F32, tag="mm")
                            for db in range(NB):
                                nc.tensor.matmul(out=p[:], lhsT=w_ap[db], rhs=xn[:, db, ss],
                                                 start=(db == 0), stop=(db == NB - 1))
                            nc.scalar.activation(qkT[:, cb, ss], p[:],
                                                 AF.Identity, bias=bqk[:, cb:cb+1])

                        def qkv_sc(sc):
                            for cb in range(8):
                                qkv_cols(cb, [wqk_all[:, cb, db, :] for db in range(NB)],
                                         sc)

                        rms(0)
                        rms(1)
                        qkv_sc(0)
                        rms(2)
                        qkv_sc(1)
                        rms(3)
                        qkv_sc(2)
                        qkv_sc(3)

                    # ============ phase B: v ============
                    with tc.tile_pool(name="ps1b", bufs=8, space="PSUM") as psb1:
                        for sb in range(NB):
                            p = psb1.tile([128, 512], F32, tag="mm")
                            for db in range(NB):
                                nc.tensor.matmul(out=p[:],
                                                 lhsT=xn[:, db, sb*128:(sb+1)*128],
                                                 rhs=wvt[:, db, :],
                                                 start=(db == 0), stop=(db == NB - 1))
                            nc.scalar.activation(vsb[:, sb, :], p[:], AF.Copy)

                # wvt pool closed here
                with tc.tile_pool(name="att", bufs=1) as attp:
                        flo = attp.tile([128, 4, 1024], BF16)
                        nc.sync.dma_start(out=flo[:], in_=flo_d[:, :, :])
                        fhi = attp.tile([128, 4, 1024], BF16)
                        nc.sync.dma_start(out=fhi[:], in_=fhi_d[:, :, :])
                        dgh = attp.tile([128, 4, 896], BF16)
                        nc.sync.dma_start(out=dgh[:], in_=dgh_d[:, :, :])
                        bexp = attp.tile([128, 4, 28], F32)
                        nc.sync.dma_start(out=bexp[:], in_=bexp_d[:, :, :])

                        # ============ phase B2: gate (streamed weights) ============
                        with tc.tile_pool(name="wgp", bufs=4) as wgp, \
                             tc.tile_pool(name="ps1c", bufs=8, space="PSUM") as psb2:
                            wgq = []
                            for cb in range(4):
                                t = wgp.tile([128, NB, 128], BF16, tag="wg")
                                nc.sync.dma_start(out=t[:], in_=wg_d[cb, :, :, :])
                                wgq.append(t)
                            for cb in range(4):
                                w = wgq.pop(0)
                                for sc in range(SC):
                                    p = psb2.tile([128, 512], F32, tag="mm")
                                    for db in range(NB):
                                        nc.tensor.matmul(out=p[:], lhsT=w[:, db, :],
                                                         rhs=xn[:, db, sc*512:(sc+1)*512],
                                                         start=(db == 0), stop=(db == NB - 1))
                                    nc.scalar.activation(gateT[:, cb, sc*512:(sc+1)*512], p[:],
                                                         AF.Sigmoid, bias=bg[:, cb:cb+1])

                        # ============ attention (software-pipelined) ============
                        with tc.tile_pool(name="wop", bufs=1) as wop:
                            # out_proj weights: issue DMAs early so they land
                            # during attention
                            woq = []
                            for ob in range(4):
                                t = wop.tile([128, 4, 128], BF16, tag="wo", bufs=4)
                                nc.sync.dma_start(out=t[:], in_=wout_d[ob, :, :, :])
                                woq.append(t)

                            with tc.tile_pool(name="attw", bufs=1) as attw, \
                                 tc.tile_pool(name="ps2", bufs=1, space="PSUM") as ps2, \
                                 tc.tile_pool(name="psA", bufs=2, space="PSUM") as psA:
                                epi = None
                                cpi = None
                                tfl = None
                                for h in range(4):
                                    for qh in range(QH):
                                        q0 = qh * 1024
                                        seq = _present_kbs(h, qh)
                                        n = len(seq)
                                        probsq = {}

                                        def emit(i, h=h, qh=qh, q0=q0, seq=seq,
                                                 probsq=probsq):
                                            kb = seq[i]
                                            sps = psA.tile([128, 1024], F32, tag="sc")
                                            probs = attw.tile([128, 1024], BF16,
                                                              tag="probs", bufs=6)
                                            for jj in range(2):
                                                js = slice(jj*512, (jj+1)*512)
                                                qa = q0 + jj * 512
                                                nc.tensor.matmul(
                                                    out=sps[:, js],
                                                    lhsT=qkT[:, 4 + h, kb*128:(kb+1)*128],
                                                    rhs=qkT[:, h, qa:qa+512],
                                                    start=True, stop=True)
                                            # class of kb vs this 1024-wide q window:
                                            # lo (fully left), md0 (diag of jj=0),
                                            # md1 (diag of jj=1), hi (fully right)
                                            base = 8 * qh
                                            if kb < base:
                                                bcol = (base - kb) - 1        # blo[d]
                                            elif kb < base + 4:
                                                m = kb - base                 # md0
                                                bcol = 16 + m
                                            elif kb < base + 8:
                                                m = kb - base - 4             # md1
                                                bcol = 8 + m
                                            else:
                                                bcol = 8 + (kb - base - 8)    # bhi[e]
                                            # mixed kbs: diag half gets exact banded
                                            # bias and cancels the exp bias (fp32)
                                            if base <= kb < base + 4:
                                                nc.vector.scalar_tensor_tensor(
                                                    out=sps[:, 0:512], in0=sps[:, 0:512],
                                                    scalar=bexp[:, h, 20+m:21+m],
                                                    in1=dgh[:, h, 384-128*m:896-128*m],
                                                    op0=ALU.add, op1=ALU.add)
                                            elif base + 4 <= kb < base + 8:
                                                nc.vector.scalar_tensor_tensor(
                                                    out=sps[:, 512:1024], in0=sps[:, 512:1024],
                                                    scalar=bexp[:, h, 24+m:25+m],
                                                    in1=dgh[:, h, 384-128*m:896-128*m],
                                                    op0=ALU.add, op1=ALU.add)
                                            nc.scalar.activation(probs[:], sps[:],
                                                                 AF.Exp,
                                                                 bias=bexp[:, h, bcol:bcol+1])
                                            # off-diag decay: per-q factor <= 1
                                            if kb < base:
                                                nc.vector.tensor_tensor(
                                                    out=probs[:], in0=probs[:],
                                                    in1=flo[:, h, :], op=ALU.mult)
                                            elif kb < base + 4:
                                                nc.vector.tensor_tensor(
                                                    out=probs[:, 512:1024],
                                                    in0=probs[:, 512:1024],
                                                    in1=flo[:, h, 0:512], op=ALU.mult)
                                            elif kb < base + 8:
                                                nc.vector.tensor_tensor(
                                                    out=probs[:, 0:512],
                                                    in0=probs[:, 0:512],
                                                    in1=fhi[:, h, 512:1024], op=ALU.mult)
                                            else:
                                                nc.vector.tensor_tensor(
                                                    out=probs[:], in0=probs[:],
                                                    in1=fhi[:, h, :], op=ALU.mult)
                                            probsq[kb] = probs

                                        # emit first tiles of this pair before the
                                        # deferred tail/cpi of the previous pair so
                                        # the PE never drains at the boundary
                                        emit(0)
                                        if tfl is not None:
                                            tfl()
                                            tfl = None
                                        if cpi is not None:
                                            cpi()
                                            cpi = None
                                        emit(1)
                                        if epi is not None:
                                            epi()
                                            epi = None

                                        ctx = ps2.tile([128, 1024], F32, tag="ctx")
                                        lps = ps2.tile([128, 1024], F32, tag="lps")

                                        def flush_win(fl, cnt=4, seq=seq, lps=lps,
                                                      ctx=ctx, h=h, probsq=probsq,
                                                      n=n):
                                            ws = seq[fl:fl + cnt]
                                            # denominator: pre-sum tile pairs on the
                                            # (otherwise idle) gpsimd, halving the
                                            # PE ones-matmul streams
                                            pos = fl
                                            while pos < fl + len(ws):
                                                a = seq[pos]
                                                if pos + 1 < n and pos + 1 < fl + len(ws):
                                                    b = seq[pos + 1]
                                                    lsum = attw.tile([128, 1024], BF16,
                                                                     tag="lsum", bufs=2)
                                                    nc.gpsimd.tensor_tensor(
                                                        out=lsum[:], in0=probsq[a][:],
                                                        in1=probsq[b][:], op=ALU.add)
                                                    rt = lsum
                                                else:
                                                    rt = probsq[a]
                                                for jj in range(2):
                                                    js = slice(jj*512, (jj+1)*512)
                                                    nc.tensor.matmul(
                                                        out=lps[:, js],
                                                        lhsT=ones128[:],
                                                        rhs=rt[:, js],
                                                        start=(pos == 0),
                                                        stop=(pos + 2 >= n))
                                                pos += 2
                                            for pk in ws:
                                                for jj in range(2):
                                                    js = slice(jj*512, (jj+1)*512)
                                                    nc.tensor.matmul(
                                                        out=ctx[:, js],
                                                        lhsT=vsb[:, pk, h*128:(h+1)*128],
                                                        rhs=probsq[pk][:, js],
                                                        start=(pk == seq[0]),
                                                        stop=(pk == seq[-1]))
                                            return len(ws)

                                        flushed = 0
                                        for i in range(2, n):
                                            emit(i)
                                            while flushed + 4 <= i - 1:
                                                flushed += flush_win(flushed)
                                        def tfl(flushed=flushed,
                                                flush_win=flush_win, n=n):
                                            while flushed < n:
                                                flushed += flush_win(flushed)

                                        # free the PSUM banks fast via ACT copies;
                                        # defer the DVE chain into the next iteration
                                        lpsS = attw.tile([128, 1024], F32, tag="lpsS", bufs=1)
                                        cu = attw.tile([128, 1024], BF16, tag="cu", bufs=1)

                                        def cpi(lps=lps, ctx=ctx, lpsS=lpsS, cu=cu):
                                            nc.scalar.activation(lpsS[:], lps[:],
                                                                 AF.Copy)
                                            nc.scalar.activation(cu[:], ctx[:], AF.Copy)

                                        def epi(h=h, q0=q0, lpsS=lpsS, cu=cu):
                                            nc.vector.reciprocal_approx_fast(out=lpsS[:],
                                                                             in_=lpsS[:])
                                            nc.vector.tensor_tensor(out=cu[:], in0=cu[:],
                                                                    in1=lpsS[:], op=ALU.mult)
                                            nc.vector.tensor_scalar(out=cu[:], in0=cu[:],
                                                                    scalar1=bv[:, h:h+1],
                                                                    scalar2=None, op0=ALU.add)
                                            nc.vector.tensor_tensor(out=gateT[:, h, q0:q0+1024],
                                                                    in0=cu[:],
                                                                    in1=gateT[:, h, q0:q0+1024],
                                                                    op=ALU.mult)
                                if tfl is not None:
                                    tfl()
                                    tfl = None
                                if cpi is not None:
                                    cpi()
                                    cpi = None
                                if epi is not None:
                                    epi()
                                    epi = None

                            # ============ out_proj (two column passes) ============
                            # prefetch first FFN up weights during out_proj
                            ffq = []
                            for L in range(3):
                                nfb, nui = divmod(L, 2)
                                t = ffw.tile([128, NB, 128], BF16, tag="wu", bufs=3)
                                nc.sync.dma_start(out=t[:], in_=wup_d[16 * nui + nfb, :, :, :])
                                ffq.append(t)

                            with tc.tile_pool(name="ps4", bufs=8, space="PSUM") as ps4:
                                for pas, scl in ((0, (0, 1)), (1, (2, 3))):
                                    for ob in range(16):
                                        w = woq.pop(0)
                                        o = evp.tile([128, 1024], BF16, tag="oevf",
                                                     bufs=4)
                                        for si, sc in enumerate(scl):
                                            p = ps4.tile([128, 512], F32, tag="mm")
                                            for cb in range(4):
                                                nc.tensor.matmul(
                                                    out=p[:], lhsT=w[:, cb, :],
                                                    rhs=gateT[:, cb, sc*512:(sc+1)*512],
                                                    start=(cb == 0), stop=(cb == 3))
                                            nc.scalar.activation(
                                                o[:, si*512:(si+1)*512], p[:], AF.Copy)
                                        nc.sync.dma_start(
                                            out=ao_d[ob, :, pas*1024:(pas+1)*1024], in_=o[:])
                                        nob = ob + 4
                                        if pas == 0 and nob >= 16:
                                            nob -= 16   # re-stream for pass 2
                                        if pas == 0 or ob + 4 < 16:
                                            t = wop.tile([128, 4, 128], BF16, tag="wo",
                                                         bufs=4)
                                            nc.sync.dma_start(out=t[:],
                                                              in_=wout_d[nob, :, :, :])
                                            woq.append(t)

            # ============ FFN (both halves), xn still resident ============
            # weight consumption order: h0 fb0-7, h1 fb0 (hoisted for overlap),
            # dw h0, h1 fb1-7, dw h1
            upseq = [16 * ui + fb for fb in range(8) for ui in range(2)]
            upseq += [16 * ui + 8 for ui in range(2)]
            upseq += [16 * ui + fb for fb in range(9, 16) for ui in range(2)]
            uptr = [3]  # first 3 already DMA'd into ffq before out_proj

            def get_w():
                w = ffq.pop(0)
                if uptr[0] < len(upseq):
                    t = ffw.tile([128, NB, 128], BF16, tag="wu", bufs=3)
                    nc.sync.dma_start(out=t[:], in_=wup_d[upseq[uptr[0]], :, :, :])
                    ffq.append(t)
                    uptr[0] += 1
                return w

            with tc.tile_pool(name="ff", bufs=1) as ffp, \
                 tc.tile_pool(name="ps5", bufs=8, space="PSUM") as ps:

                def up_block(fb, hsb, fbi):
                    u = [None, None]
                    for ui in range(2):
                        w = get_w()
                        ut = ffp.tile([128, S], BF16, tag=f"u{ui}", bufs=2)
                        for sc in range(SC):
                            p = ps.tile([128, 512], F32, tag="mm")
                            for db in range(NB):
                                nc.tensor.matmul(out=p[:], lhsT=w[:, db, :],
                                                 rhs=xn[:, db, sc*512:(sc+1)*512],
                                                 start=(db == 0), stop=(db == NB - 1))
                            func = AF.Silu if ui == 0 else AF.Identity
                            nc.scalar.activation(ut[:, sc*512:(sc+1)*512], p[:], func,
                                                 bias=bup[:, 16*ui+fb:16*ui+fb+1])
                        u[ui] = ut
                    nc.vector.tensor_tensor(out=hsb[:, fbi, :], in0=u[0][:], in1=u[1][:],
                                            op=ALU.mult)

                def dw_block(half, hsb, od):
                    for ob in range(16):
                        w = ffw.tile([128, 8, 128], BF16, tag="wdw", bufs=3)
                        nc.sync.dma_start(out=w[:], in_=wdw_d[ob, half, :, :, :])
                        o = ffp.tile([128, S], BF16, tag="ofat", bufs=2)
                        for sc in range(SC):
                            p = ps.tile([128, 512], F32, tag="mm")
                            for fbi in range(8):
                                nc.tensor.matmul(out=p[:], lhsT=w[:, fbi, :],
                                                 rhs=hsb[:, fbi, sc*512:(sc+1)*512],
                                                 start=(fbi == 0), stop=(fbi == 7))
                            nc.scalar.activation(o[:, sc*512:(sc+1)*512], p[:], AF.Copy)
                        nc.sync.dma_start(out=od[ob, :, :], in_=o[:])

                hsbA = ffp.tile([128, 8, S], BF16, tag="hsb", bufs=2)
                for fbi in range(8):
                    up_block(fbi, hsbA, fbi)
                hsbB = ffp.tile([128, 8, S], BF16, tag="hsb", bufs=2)
                up_block(8, hsbB, 0)
                dw_block(0, hsbA, fa_d)
                for fbi in range(1, 8):
                    up_block(8 + fbi, hsbB, fbi)
                dw_block(1, hsbB, fb_d)

    nc.compile()
    return nc


def _slopes():
    start = 2.0 ** (-8.0 / H)
    return np.array([start ** (i + 1) for i in range(H)], dtype=np.float32)


def _host_shard(inputs):
    x = np.asarray(inputs["x"], np.float32)
    rms_w = np.asarray(inputs["rms_w"], np.float32)
    qkv_w = np.asarray(inputs["qkv_w"], np.float32) * rms_w[:, None]
    qkv_b = np.asarray(inputs["qkv_b"], np.float32)
    up_w = np.asarray(inputs["up_w"], np.float32) * rms_w[:, None]
    up_b = np.asarray(inputs["up_b"], np.float32)
    dw_w = np.asarray(inputs["dw_w"], np.float32)
    gate_w = np.asarray(inputs["gate_w"], np.float32) * rms_w[:, None]
    gate_b = np.asarray(inputs["gate_b"], np.float32)
    out_w = np.asarray(inputs["out_w"], np.float32)
    slopes = np.asarray(inputs["alibi_slopes"], np.float32)
    sc = 1.0 / math.sqrt(DH)
    idx = np.arange(S, dtype=np.float32)

    in_maps = []
    for c in range(8):
        g, j = c // 4, c % 4
        # slot t holds head 4t+j: every core gets one head per slope quartile,
        # so the static block-skip pattern (keyed on the slot's minimum slope)
        # is valid on all cores
        hds = [4 * t + j for t in range(4)]
        qcols = np.concatenate([np.arange(128 * hh, 128 * hh + 128) for hh in hds])
        fc = slice(2048 * j, 2048 * j + 2048)

        wq = qkv_w[:, qcols] * sc
        wk = qkv_w[:, 2048 + qcols]
        wqk = np.concatenate([wq, wk], 1)                     # [2048,1024]
        wqk_h = wqk.reshape(NB, 128, 8, 128).transpose(2, 1, 0, 3).astype(bf)
        wv = qkv_w[:, 4096 + qcols]
        wv_h = wv.reshape(NB, 128, 512).transpose(1, 0, 2).astype(bf)
        wg_h = gate_w[:, qcols].reshape(NB, 128, 4, 128).transpose(2, 1, 0, 3).astype(bf)
        wup = np.concatenate([up_w[:, fc], up_w[:, DF + 2048*j: DF + 2048*j + 2048]], 1)
        wup_h = wup.reshape(NB, 128, 32, 128).transpose(2, 1, 0, 3).astype(bf)
        wdw_h = dw_w[fc, :].reshape(2, 8, 128, 16, 128).transpose(3, 0, 2, 1, 4).astype(bf)
        wout_h = out_w[qcols, :].reshape(4, 128, 16, 128).transpose(2, 1, 0, 3).astype(bf)

        bq = qkv_b[qcols] * sc
        bk = qkv_b[2048 + qcols]
        bqk_h = np.concatenate([bq, bk]).reshape(8, 128).T.astype(np.float32).copy()
        bg_h = gate_b[qcols].reshape(4, 128).T.astype(np.float32).copy()
        bup_h = np.concatenate([up_b[fc], up_b[DF + 2048*j: DF + 2048*j + 2048]]
                               ).reshape(32, 128).T.astype(np.float32).copy()
        bv_h = qkv_b[4096 + qcols].reshape(4, 128).T.astype(np.float32).copy()

        dgh = np.zeros((128, 4, 896), np.float32)
        flo = np.zeros((128, 4, 1024), np.float32)
        fhi = np.zeros((128, 4, 1024), np.float32)
        bexp = np.zeros((128, 4, 28), np.float32)
        p = np.arange(128, dtype=np.float32)
        t1024 = np.arange(1024, dtype=np.float32)
        for t, hh in enumerate(hds):
            s = slopes[hh]
            jx = np.arange(896)[None, :]
            dgh[:, t, :] = -s * np.abs(jx - 384 - p[:, None])
            flo[:, t, :] = np.exp(-s * t1024)[None, :]
            fhi[:, t, :] = np.exp(-s * (1023.0 - t1024))[None, :]
            for d in range(1, 9):
                bexp[:, t, d - 1] = s * (p - 128.0 * d)
            for e in range(8):
                bexp[:, t, 8 + e] = -s * (128.0 * e + p + 1.0)
            for m in range(4):
                bexp[:, t, 16 + m] = s * (128.0 * m + p - 512.0)
                bexp[:, t, 20 + m] = -s * (128.0 * m + p - 512.0)
                bexp[:, t, 24 + m] = s * (128.0 * m + p + 1.0)

        xT_h = x[g].T.reshape(NB, 128, S).astype(bf)

        in_maps.append({
            "xT": np.ascontiguousarray(xT_h),
            "wqk": np.ascontiguousarray(wqk_h), "wv": np.ascontiguousarray(wv_h),
            "wg": np.ascontiguousarray(wg_h), "wup": np.ascontiguousarray(wup_h),
            "wdw": np.ascontiguousarray(wdw_h), "wout": np.ascontiguousarray(wout_h),
            "flo": flo.astype(bf), "fhi": fhi.astype(bf),
            "dgh": np.ascontiguousarray(dgh).astype(bf),
            "bexp": bexp,
            "bqk": bqk_h, "bg": bg_h, "bup": bup_h, "bv": bv_h,
        })
    return in_maps


def kernel(**inputs):
    global _NC
    if _NC is None:
        _NC = _build()
    in_maps = _host_shard(inputs)
    trace = os.environ.get("BASS_KERNEL_TRACE") == "1"
    res = run_bass_kernel_spmd(_NC, in_maps, list(range(8)), trace=trace)
    global LAST_EXEC_NS
    LAST_EXEC_NS = res.exec_time_ns
    out_b = np.asarray(inputs["out_b"], np.float32)
    dw_b = np.asarray(inputs["dw_b"], np.float32)
    out = np.zeros((2, S, D), np.float32)
    for c in range(8):
        g = c // 4
        r = res.results[c]
        for k in ("attn_outT", "ffn_aT", "ffn_bT"):
            out[g] += r[k].astype(np.float32).reshape(D, S).T
    out += out_b + dw_b
    return out

